# revision 11
# baseline (speedup 1.0000x reference)
"""Trainium2 Bass kernel for nn_Actor (pointer-network actor: encoder LSTM +
greedy attention pointer decoder). 8-core pure data parallelism over batch.

kernel(**inputs) takes FULL inputs (B=512) and returns (tour[i32], log_probability, h)
matching reference.reference().
"""
import sys
sys.path.insert(0, "/opt/trn_rl_repo")
import numpy as np
from contextlib import ExitStack

import concourse.bass as bass
import concourse.tile as tile
from concourse import bacc, mybir
from concourse.bass_utils import run_bass_kernel_spmd

F32 = mybir.dt.float32
U32 = mybir.dt.uint32
AF = mybir.ActivationFunctionType
ALU = mybir.AluOpType
AX = mybir.AxisListType

B, S, D, H = 512, 100, 2, 256
NCORES = 8
BL = B // NCORES          # 64 rows per core
NEG = -1.0e30

IN_SPECS = [("xT3", [3, BL * S]), ("xd", [BL, 2 * S]),
            ("wxb_e", [3, 1024]), ("wxb_d", [3, 1024]),
            ("whhT_e", [128, 2048]), ("whhT_d", [128, 2048]),
            ("weT", [128, 512]), ("wdT", [128, 512]),
            ("vv", [128, 2]), ("be2", [128, 2]),
            ("startT3", [3, BL]), ("iota", [BL, S]), ("ident", [128, 128])]


def build(nc, ENC_STEPS=S, DEC_STEPS=S):
    P = {}
    for nm, shp in IN_SPECS:
        P[nm] = nc.declare_dram_parameter(nm, list(shp), F32, isOutput=False)
    for nm, shp in [("tour_f", [BL, S]), ("lp", [BL, 1]), ("hout", [BL, H])]:
        P[nm] = nc.declare_dram_parameter(nm, list(shp), F32, isOutput=True)

    with tile.TileContext(nc, trace_sim=False) as tc, ExitStack() as ctx:
        # ---- pools (careful SBUF budget; per-partition bytes shown) ----
        wpool = ctx.enter_context(tc.tile_pool(name="w", bufs=1))       # weights (no xT3): ~32KB
        state = ctx.enter_context(tc.tile_pool(name="st", bufs=1))      # small states ~4KB
        sc_pool = ctx.enter_context(tc.tile_pool(name="scp", bufs=2))   # ~18KB
        e_pool = ctx.enter_context(tc.tile_pool(name="ep", bufs=1))     # e0,e1 51.2KB
        ps_g = ctx.enter_context(tc.tile_pool(name="psg", bufs=1, space="PSUM"))
        ps_t = ctx.enter_context(tc.tile_pool(name="pst", bufs=2, space="PSUM"))
        ps_s = ctx.enter_context(tc.tile_pool(name="pss", bufs=2, space="PSUM"))

        w = {}
        for nm, shp in IN_SPECS:
            if nm == "xT3":
                continue
            tl = wpool.tile(shp, F32, tag=nm)
            nc.sync.dma_start(tl[:], P[nm].ap())
            w[nm] = tl

        e0 = e_pool.tile([128, BL * S], F32, tag="e0")
        e1 = e_pool.tile([128, BL * S], F32, tag="e1")
        e_ = [e0, e1]
        ctil = state.tile([BL, H], F32, tag="ctil")       # c~ = 2c
        htil = state.tile([BL, H], F32, tag="htil")       # h~ = 2h
        hT = state.tile([128, 128], F32, tag="hT")        # h~^T: col kc*64+b
        mask = state.tile([BL, S], F32, tag="mask")
        dinT3 = state.tile([3, BL], F32, tag="dinT3")
        tour_f = state.tile([BL, S], F32, tag="tourf")
        sumexp = state.tile([BL, S], F32, tag="sumexp")
        nc.vector.memset(ctil[:], 0.0)
        nc.vector.memset(mask[:], 0.0)
        nc.vector.tensor_copy(dinT3[0:3, :], w["startT3"][0:3, :])

        def lstm_step(t, wxb, whhT, x_lhsT, h_allT=None, first=False):
            gp = ps_g.tile([BL, 1024], F32, tag="gates")
            for nb in range(2):
                nc.tensor.matmul(gp[:, nb * 512:(nb + 1) * 512], x_lhsT,
                                 wxb[:, nb * 512:(nb + 1) * 512],
                                 start=True, stop=first)
            if not first:
                for kc in range(2):
                    for nb in range(2):
                        nc.tensor.matmul(gp[:, nb * 512:(nb + 1) * 512],
                                         hT[:, kc * 64:kc * 64 + BL],
                                         whhT[:, kc * 1024 + nb * 512: kc * 1024 + (nb + 1) * 512],
                                         start=False, stop=(kc == 1))
            # gate cols: [i(256) f(256) o(256) g(256)]; sig(z)=(1+tanh(z/2))/2
            tio = sc_pool.tile([BL, 768], F32, tag="tio")
            nc.scalar.activation(tio[:], gp[:, 0:768], AF.Tanh, scale=0.5)
            tg = sc_pool.tile([BL, 256], F32, tag="tg")
            nc.scalar.activation(tg[:], gp[:, 768:1024], AF.Tanh)
            aa = sc_pool.tile([BL, 256], F32, tag="aa")
            nc.vector.scalar_tensor_tensor(aa[:], tio[:, 256:512], 1.0, ctil[:],
                                           op0=ALU.add, op1=ALU.mult)
            bb = sc_pool.tile([BL, 256], F32, tag="bb")
            nc.vector.scalar_tensor_tensor(bb[:], tio[:, 0:256], 1.0, tg[:],
                                           op0=ALU.add, op1=ALU.mult)
            nc.vector.scalar_tensor_tensor(ctil[:], aa[:], 0.5, bb[:],
                                           op0=ALU.mult, op1=ALU.add)
            tcn = sc_pool.tile([BL, 256], F32, tag="tcn")
            nc.scalar.activation(tcn[:], ctil[:], AF.Tanh, scale=0.5)
            nc.vector.scalar_tensor_tensor(htil[:], tio[:, 512:768], 1.0, tcn[:],
                                           op0=ALU.add, op1=ALU.mult)
            for kc in range(2):
                tp = ps_t.tile([128, 128], F32, tag="tp")
                nc.tensor.transpose(tp[0:128, 0:64], htil[:, kc * 128:(kc + 1) * 128],
                                    w["ident"][0:BL, 0:BL])
                nc.scalar.copy(hT[:, kc * 64:(kc + 1) * 64], tp[0:128, 0:64])
                if h_allT is not None:
                    nc.vector.tensor_copy(h_allT[:, t * 128 + kc * 64: t * 128 + (kc + 1) * 64],
                                          tp[0:128, 0:64])

        # ================= encoder =================
        with tc.tile_pool(name="enc", bufs=1) as encpool:
            h_allT = encpool.tile([128, S * 128], F32, tag="hallT")
            with tc.tile_pool(name="xp", bufs=1) as xpool:
                xT3 = xpool.tile([3, BL * S], F32, tag="xT3")
                nc.sync.dma_start(xT3[:], P["xT3"].ap())
                for t in range(ENC_STEPS):
                    xl = xT3[:, :].rearrange("p (b s) -> p s b", s=S)[:, t, :]
                    lstm_step(t, w["wxb_e"], w["whhT_e"], xl, h_allT=h_allT, first=(t == 0))

            # e build
            EB = 4
            for hc in range(2):
                for c0 in range(0, BL, EB):
                    pe128 = ps_s.tile([128, EB * S], F32, tag="vp")
                    for kc in range(2):
                        rhs = h_allT[:, :].rearrange("p (t kc b) -> p kc b t", kc=2, b=64)[:, kc, c0:c0 + EB, :]
                        nc.tensor.matmul(pe128[:], w["weT"][:, (kc * 2 + hc) * 128:(kc * 2 + hc + 1) * 128],
                                         rhs, start=(kc == 0), stop=(kc == 1))
                    nc.scalar.activation(e_[hc][:, c0 * S:(c0 + EB) * S], pe128[:], AF.Identity,
                                         bias=w["be2"][:, hc:hc + 1])

        # ================= decoder =================
        with tc.tile_pool(name="up", bufs=2) as upool, \
             tc.tile_pool(name="sct", bufs=1) as sctp:
            scoresT = sctp.tile([128, 16 * S], F32, tag="scoresT")
            for t in range(DEC_STEPS):
                lstm_step(t, w["wxb_d"], w["whhT_d"], dinT3[:, :], h_allT=None, first=False)
                # dec_proj dT [128, (hc,b)]
                dt_ps = ps_t.tile([128, 128], F32, tag="tp")
                for hc in range(2):
                    for kc in range(2):
                        nc.tensor.matmul(dt_ps[:, hc * 64:hc * 64 + BL],
                                         w["wdT"][:, (kc * 2 + hc) * 128:(kc * 2 + hc + 1) * 128],
                                         hT[:, kc * 64:kc * 64 + BL],
                                         start=(kc == 0), stop=(kc == 1))
                dT = sc_pool.tile([128, 128], F32, tag="dTsb")
                nc.scalar.copy(dT[:], dt_ps[:])

                scores = sc_pool.tile([BL, S], F32, tag="scores")
                NSUB = 2
                SUBB = BL // NSUB
                cp_i = 0
                for sub in range(NSUB):
                    us = []
                    for hc in range(2):
                        upre = upool.tile([128, SUBB * S], F32, tag="upre")
                        dslc = dT[:, hc * 64 + sub * SUBB: hc * 64 + (sub + 1) * SUBB]
                        nc.vector.tensor_tensor(
                            upre[:].rearrange("p (b s) -> p b s", s=S),
                            e_[hc][:, sub * SUBB * S:(sub + 1) * SUBB * S].rearrange("p (b s) -> p b s", s=S),
                            dslc[:, :, None].broadcast_to((128, SUBB, S)),
                            op=ALU.add)
                        ut = upool.tile([128, SUBB * S], F32, tag="ut")
                        nc.scalar.activation(ut[:], upre[:], AF.Tanh)
                        us.append(ut)
                    # V-reduce into psum [1,800] rows -> copy to scoresT row
                    NS = SUBB * S  # 3200
                    for p0 in range(0, NS, 800):
                        vp = ps_s.tile([1, 1024], F32, tag="vp")
                        for q0 in (0, 512):
                            qw = 512 if q0 == 0 else 288
                            for hc in range(2):
                                nc.tensor.matmul(vp[0:1, q0:q0 + qw], w["vv"][:, hc:hc + 1],
                                                 us[hc][:, p0 + q0:p0 + q0 + qw],
                                                 start=(hc == 0), stop=(hc == 1))
                        off = sub * NS + p0          # multiple of 800 in [0, 6400)
                        row = 32 * (off // 1600)
                        col = off % 1600
                        dst = scoresT[row:row + 1, col:col + 800]
                        if cp_i % 2 == 0:
                            nc.scalar.copy(dst, vp[0:1, 0:800])
                        else:
                            nc.vector.tensor_copy(dst, vp[0:1, 0:800])
                        cp_i += 1
                nc.sync.dma_start(
                    scores[:, :],
                    scoresT[0:128:32, :].rearrange("p (b s) -> p b s", s=S))

                # mask + argmax + bookkeeping
                ms = sc_pool.tile([BL, S], F32, tag="ms")
                nc.vector.tensor_tensor(ms[:], scores[:], mask[:], op=ALU.add)
                mx = sc_pool.tile([BL, 8], F32, tag="mx")
                nc.vector.max(mx[:], ms[:])
                ix = sc_pool.tile([BL, 8], U32, tag="ix")
                nc.vector.max_index(ix[:], mx[:], ms[:])
                sel_f = sc_pool.tile([BL, 1], F32, tag="selF")
                nc.vector.tensor_copy(sel_f[:], ix[:, 0:1])
                nc.vector.tensor_copy(tour_f[:, t:t + 1], ix[:, 0:1])
                nmx = sc_pool.tile([BL, 1], F32, tag="nmx")
                nc.vector.tensor_scalar(nmx[:], mx[:, 0:1], -1.0, None, op0=ALU.mult)
                exs = sc_pool.tile([BL, S], F32, tag="exs")
                nc.scalar.activation(exs[:], ms[:], AF.Exp, bias=nmx[:, 0:1],
                                     accum_out=sumexp[:, t:t + 1])
                oh = sc_pool.tile([BL, S], F32, tag="oh")
                nc.vector.tensor_scalar(oh[:], w["iota"][:], sel_f[:, 0:1], None, op0=ALU.is_equal)
                nc.vector.scalar_tensor_tensor(mask[:], oh[:], NEG, mask[:], op0=ALU.mult, op1=ALU.add)
                if t < S - 1:
                    din2 = sc_pool.tile([BL, 2], F32, tag="din2")
                    dsc = sc_pool.tile([BL, S], F32, tag="dsc")
                    for dd in range(2):
                        nc.vector.scalar_tensor_tensor(dsc[:], oh[:], 1.0,
                                                       w["xd"][:, dd * S:(dd + 1) * S],
                                                       op0=ALU.mult, op1=ALU.mult,
                                                       accum_out=din2[:, dd:dd + 1])
                    dtp = ps_t.tile([128, 128], F32, tag="tp")
                    nc.tensor.transpose(dtp[0:2, 0:BL], din2[:], w["ident"][0:BL, 0:BL])
                    nc.vector.tensor_copy(dinT3[0:2, :], dtp[0:2, 0:BL])

        # ---- outputs ----
        lg = sc_pool.tile([BL, S], F32, tag="lg")
        nc.scalar.activation(lg[:], sumexp[:], AF.Ln)
        lps = sc_pool.tile([BL, 1], F32, tag="lps")
        nc.vector.tensor_reduce(lps[:], lg[:], axis=AX.X, op=ALU.add)
        lpn = sc_pool.tile([BL, 1], F32, tag="lpn")
        nc.vector.tensor_scalar(lpn[:], lps[:], -1.0, None, op0=ALU.mult)
        hhalf = sc_pool.tile([BL, H], F32, tag="hhalf")
        nc.scalar.mul(hhalf[:], htil[:], 0.5)
        nc.sync.dma_start(P["tour_f"].ap(), tour_f[:])
        nc.sync.dma_start(P["lp"].ap(), lpn[:])
        nc.sync.dma_start(P["hout"].ap(), hhalf[:])
    return nc


# ---------------- host-side prep ----------------

def _perm_cols(w4h):
    """[4H, ...] torch gate rows (i,f,g,o) -> (i,f,o,g) order."""
    i, f, g, o = np.split(w4h, 4, axis=0)
    return np.concatenate([i, f, o, g], axis=0)


def _prep_weights(inp):
    out = {}
    for pre, WihK, WhhK, bihK, bhhK in [("e", "enc_Wih", "enc_Whh", "enc_bih", "enc_bhh"),
                                        ("d", "dec_Wih", "dec_Whh", "dec_bih", "dec_bhh")]:
        Wih = _perm_cols(np.asarray(inp[WihK], np.float32))        # [1024, 2]
        bias = _perm_cols((np.asarray(inp[bihK], np.float32) +
                           np.asarray(inp[bhhK], np.float32))[:, None])[:, 0]
        out[f"wxb_{pre}"] = np.concatenate([Wih.T, bias[None, :]], axis=0).astype(np.float32)
        Whh = _perm_cols(np.asarray(inp[WhhK], np.float32)) * 0.5  # [1024, 256]
        WhhT = Whh.T  # [256, 1024]
        out[f"whhT_{pre}"] = np.ascontiguousarray(
            WhhT.reshape(2, 128, 1024).transpose(1, 0, 2).reshape(128, 2048)).astype(np.float32)
    for nm, WK in [("weT", "att_We"), ("wdT", "att_Wd")]:
        W = np.asarray(inp[WK], np.float32) * 0.5   # [256(h'), 256(k)]
        WT = W.T                                     # [k, h']
        blocks = [WT[kc * 128:(kc + 1) * 128, hc * 128:(hc + 1) * 128]
                  for kc in range(2) for hc in range(2)]
        out[nm] = np.concatenate(blocks, axis=1).astype(np.float32)
    V = np.asarray(inp["V_w"], np.float32)
    out["vv"] = np.stack([V[0:128], V[128:256]], axis=1).astype(np.float32)
    be = np.asarray(inp["att_be"], np.float32) + np.asarray(inp["att_bd"], np.float32)
    out["be2"] = np.stack([be[0:128], be[128:256]], axis=1).astype(np.float32)
    out["ident"] = np.eye(128, dtype=np.float32)
    return out


_CACHE = {}

def _get_runner():
    if "nc" not in _CACHE:
        nc = bacc.Bacc("TRN2", target_bir_lowering=False, debug=False, num_devices=NCORES)
        build(nc)
        nc.compile()
        _CACHE["nc"] = nc
    return _CACHE["nc"]


def kernel(**inputs):
    nc = _get_runner()
    wmaps = _prep_weights(inputs)
    x = np.asarray(inputs["x"], np.float32)
    start = np.asarray(inputs["start"], np.float32)
    startT3 = np.concatenate([np.tile(start[:, None], (1, BL)),
                              np.ones((1, BL), np.float32)], axis=0)
    iota = np.tile(np.arange(S, dtype=np.float32), (BL, 1))
    in_maps = []
    for c in range(NCORES):
        xs = x[c * BL:(c + 1) * BL]
        xT3 = np.concatenate([xs.reshape(BL * S, 2).T, np.ones((1, BL * S), np.float32)], axis=0)
        xdm = np.ascontiguousarray(xs.transpose(0, 2, 1)).reshape(BL, 2 * S)
        m = dict(wmaps)
        m["xT3"] = np.ascontiguousarray(xT3.astype(np.float32))
        m["xd"] = xdm.astype(np.float32)
        m["startT3"] = startT3
        m["iota"] = iota
        in_maps.append(m)
    res = run_bass_kernel_spmd(nc, in_maps, core_ids=list(range(NCORES)))
    tour = np.concatenate([r["tour_f"] for r in res.results], axis=0)
    lp = np.concatenate([r["lp"][:, 0] for r in res.results], axis=0)
    h = np.concatenate([r["hout"] for r in res.results], axis=0)
    return np.rint(tour).astype(np.int32), lp.astype(np.float32), h.astype(np.float32)
